# revision 42
# baseline (speedup 1.0000x reference)
"""Causal self-attention (B=2, T=2048, E=1024, H=16, d_k=64) on 8 TRN2 cores.

Hybrid sharding: core c owns batch c//4 and head group c%4 (4 heads =
feature slice 256*(c%4) .. 256*(c%4+1)).  Each core computes a partial
output [2048, 1024] for its batch; the host sums 4 partials per batch
and adds bo.

All matmuls run in bfloat16 (fp32 PSUM accumulation).  Per 512-token
chunk j the kernel pipelines: QKV projection (PE-dense), attention
(ACT-dense: one exp instruction covers both heads of a row-packed
QK pair), then the output projection.  Causal structure is exploited
by narrowing diagonal score tiles to >=their valid query range and
applying a 128x128 triangular 0/1 mask by multiply-after-exp.  The
softmax denominator comes from a ones column appended to V (computed
by the same ones-row matmul that adds the V bias).
"""

import numpy as np

B = 2
T = 2048          # tokens per batch (= per core)
E = 1024
F = 256           # per-core QKV features (4 heads x 64)
DK = 64
NH_LOC = 4        # heads per core
N_CORES = 8
IC = 512          # query chunk
JC = 128          # key chunk
NJ = T // IC      # 512-token chunks
N_EC = E // 128   # contraction chunks
FS = F // 128     # feature slices (partition groups)
VW = NH_LOC * 66  # padded V width: per head 64 feats + ones col + pad

_CACHE = {}


def _build_program(debug_taps=False):
    import concourse.mybir as mybir
    import concourse.tile as tile
    from concourse import bacc

    f32 = mybir.dt.float32
    bf = mybir.dt.bfloat16
    Act = mybir.ActivationFunctionType

    nc = bacc.Bacc("TRN2", target_bir_lowering=False, debug=False)

    f8 = mybir.dt.float8e4
    xT_ap = nc.dram_tensor("xT", [E, T], bf, kind="ExternalInput").ap()
    x8_ap = nc.dram_tensor("x8", [E, T], f8, kind="ExternalInput").ap()
    wq_ap = nc.dram_tensor("wq", [E, F], f8, kind="ExternalInput").ap()
    wk_ap = nc.dram_tensor("wk", [E, F], f8, kind="ExternalInput").ap()
    wv_ap = nc.dram_tensor("wv", [E, VW], bf, kind="ExternalInput").ap()
    wo_ap = nc.dram_tensor("wo", [F, E], bf, kind="ExternalInput").ap()
    bq_ap = nc.dram_tensor("bq", [F], f32, kind="ExternalInput").ap()
    bk_ap = nc.dram_tensor("bk", [F], f32, kind="ExternalInput").ap()
    bvr_ap = nc.dram_tensor("bvr", [1, VW], bf, kind="ExternalInput").ap()
    tril_ap = nc.dram_tensor("tril", [JC, JC], bf, kind="ExternalInput").ap()
    out_ap = nc.dram_tensor("partial", [T, E], f32, kind="ExternalOutput").ap()
    if debug_taps:
        dbg_qt = nc.dram_tensor("dbg_qt", [128, FS, T], bf, kind="ExternalOutput").ap()
        dbg_kt = nc.dram_tensor("dbg_kt", [128, FS, T], bf, kind="ExternalOutput").ap()
        dbg_v1 = nc.dram_tensor("dbg_v1", [128, T // JC, VW], bf, kind="ExternalOutput").ap()
        dbg_yt = nc.dram_tensor("dbg_yt", [128, FS, T], bf, kind="ExternalOutput").ap()

    with tile.TileContext(nc) as tc:
        with (
            tc.tile_pool(name="const", bufs=1) as constp,
            tc.tile_pool(name="persist", bufs=1) as persist,
            tc.tile_pool(name="xt", bufs=2) as xtp,
            tc.tile_pool(name="pt", bufs=4) as ptp,
            tc.tile_pool(name="work", bufs=3) as work,
            tc.tile_pool(name="ob", bufs=3) as obp,
            tc.tile_pool(name="ps", bufs=1, space="PSUM") as psp,
        ):
            # ---- constants (wq first: the first projection needs only it
            # and x chunk 0; the x load goes on the scalar HWDGE ring so it
            # runs in parallel with the weight loads on the SP ring) ----
            # q/k projection weights in fp8 (DoubleRow: 2 contraction
            # k-tiles of 128 per pass -> [p, pass, ktile, f])
            wq_sb = constp.tile([128, N_EC // 2, 2, F], f8, tag="wq")
            nc.sync.dma_start(
                wq_sb[:], wq_ap.rearrange("(a i p) f -> p a i f", p=128, i=2)
            )
            bq_sb = constp.tile([128, FS], f32, tag="bq")
            nc.sync.dma_start(bq_sb[:], bq_ap.rearrange("(s p) -> p s", p=128))
            wk_sb = constp.tile([128, N_EC // 2, 2, F], f8, tag="wk")
            nc.sync.dma_start(
                wk_sb[:], wk_ap.rearrange("(a i p) f -> p a i f", p=128, i=2)
            )
            bk_sb = constp.tile([128, FS], f32, tag="bk")
            nc.sync.dma_start(bk_sb[:], bk_ap.rearrange("(s p) -> p s", p=128))
            wv_sb = constp.tile([128, N_EC, VW], bf, tag="wv")
            nc.sync.dma_start(wv_sb[:], wv_ap.rearrange("(a p) f -> p a f", p=128))
            bvr_sb = constp.tile([1, VW], bf, tag="bvr")
            nc.sync.dma_start(bvr_sb[:], bvr_ap)
            tril_sb = constp.tile([128, JC], bf, tag="tril")
            nc.sync.dma_start(tril_sb[:], tril_ap)
            wo_sb = constp.tile([128, FS, E], bf, tag="wo")
            nc.sync.dma_start(wo_sb[:], wo_ap.rearrange("(s p) e -> p s e", p=128))
            ones_r = constp.tile([1, JC], bf, tag="ones_r")
            nc.vector.memset(ones_r[:], 1.0)
            ones64 = constp.tile([1, DK], bf, tag="ones64")
            nc.vector.memset(ones64[:], 1.0)
            ones_f32 = constp.tile([128, 1], f32, tag="ones_f32")
            nc.vector.memset(ones_f32[:], 1.0)
            ones_row = ones_f32[:, 0:1].broadcast_to([128, IC])

            # ---- persistent activations ----
            qt_sb = persist.tile([128, FS, T], bf, tag="qt")      # [f, fs, t]
            kt_sb = persist.tile([128, FS, T], bf, tag="kt")
            v1_sb = persist.tile([128, T // JC, VW], bf, tag="v1")  # [t%128, kc, hf]
            yt_sb = persist.tile([128, FS, T], bf, tag="yt")

            xre = xT_ap.rearrange("(a p) t -> p a t", p=128)
            x8re = x8_ap.rearrange("(a i p) t -> p a i t", p=128, i=2)
            xts = [None] * NJ
            x8ts = [None] * NJ

            def load_x(j):
                # fp8 copy feeds the q/k projections (DoubleRow)
                x8t = xtp.tile([128, N_EC // 2, 2, IC], f8, tag="x8t", name=f"x8t{j}")
                nc.scalar.dma_start(x8t[:], x8re[:, :, :, j * IC : (j + 1) * IC])
                x8ts[j] = x8t
                # bf16 copy feeds the V projection
                xt = xtp.tile([128, N_EC, IC], bf, tag="xt", name=f"xt{j}")
                step = 4
                for e0 in range(0, N_EC, step):
                    nc.scalar.dma_start(
                        xt[:, e0 : e0 + step, :],
                        xre[:, e0 : e0 + step, j * IC : (j + 1) * IC],
                    )
                xts[j] = xt

            def b_emitters(j):
                """Per-psum-group emission closures for chunk j's QKV
                projection.  Interleaved into the previous chunk's
                (ACT-bound) attention phase to fill PE idle time."""
                t0 = j * IC
                xt = xts[j]
                x8t = x8ts[j]
                ems = []
                for w_sb, b_sb, dst in (
                    (wq_sb, bq_sb, qt_sb),
                    (wk_sb, bk_sb, kt_sb),
                ):
                    for fs in range(FS):
                        def em(w_sb=w_sb, b_sb=b_sb, dst=dst, fs=fs):
                            pq = psp.tile([128, IC], f32, tag="aux", bufs=2)
                            for a in range(N_EC // 2):
                                nc.tensor.matmul(
                                    pq[:],
                                    w_sb[:, a, :, fs * 128 : (fs + 1) * 128],
                                    x8t[:, a, :, :],
                                    start=(a == 0),
                                    stop=(a == N_EC // 2 - 1),
                                    perf_mode=mybir.MatmulPerfMode.DoubleRow,
                                )
                            # bias-add + fp32->bf16 move on DVE (ACT is the
                            # attention-phase bottleneck; keep it exp-only)
                            nc.vector.scalar_tensor_tensor(
                                dst[:, fs, t0 : t0 + IC], pq[:],
                                b_sb[:, fs : fs + 1], ones_row[:],
                                op0=mybir.AluOpType.add,
                                op1=mybir.AluOpType.mult,
                            )
                        ems.append(em)
                # V in [token, feat] layout; ones-row matmul adds bias AND
                # writes the per-head ones column (bvr has 1.0 there).
                for tsub in range(IC // 128):
                    def em(tsub=tsub):
                        pv = psp.tile([128, VW], f32, tag="aux", bufs=2)
                        for ec in range(N_EC):
                            nc.tensor.matmul(
                                pv[:],
                                xt[:, ec, tsub * 128 : (tsub + 1) * 128],
                                wv_sb[:, ec, :],
                                start=(ec == 0),
                                stop=False,
                            )
                        nc.tensor.matmul(
                            pv[:], ones_r[:], bvr_sb[:], start=False, stop=True
                        )
                        nc.vector.tensor_copy(v1_sb[:, j * 4 + tsub, :], pv[:])
                    ems.append(em)
                return ems

            def d_emitters(j, tail=False):
                """Per-128-token output-projection closures for chunk j.
                Interleaved into the NEXT chunk's attention phase; the final
                chunk's run at the end uses the idle ACT engine + HWDGE ring
                to shorten the drain tail."""
                t0 = j * IC
                ems = []
                for tsub in range(IC // 128):
                    def em(tsub=tsub):
                        tt = t0 + tsub * 128
                        ob = obp.tile([128, E], f32, tag="ob")
                        for eo in range(2):
                            od = psp.tile([128, 512], f32, tag="aux", bufs=2)
                            for fs in range(FS):
                                nc.tensor.matmul(
                                    od[:],
                                    yt_sb[:, fs, tt : tt + 128],
                                    wo_sb[:, fs, eo * 512 : (eo + 1) * 512],
                                    start=(fs == 0),
                                    stop=(fs == FS - 1),
                                )
                            cp_eng = nc.scalar if (tail and eo == 1) else nc.vector
                            if cp_eng is nc.scalar:
                                cp_eng.copy(ob[:, eo * 512 : (eo + 1) * 512], od[:])
                            else:
                                cp_eng.tensor_copy(
                                    ob[:, eo * 512 : (eo + 1) * 512], od[:]
                                )
                            if tail:
                                # split + HWDGE: drain the last chunk fast
                                nc.sync.dma_start(
                                    out_ap[tt : tt + 128, eo * 512 : (eo + 1) * 512],
                                    ob[:, eo * 512 : (eo + 1) * 512],
                                )
                        if not tail:
                            # ride the idle GpSimd SWDGE queue mid-kernel
                            nc.gpsimd.dma_start(out_ap[tt : tt + 128, :], ob[:])
                    ems.append(em)
                return ems

            load_x(0)
            for em in b_emitters(0):
                em()
            for j in range(NJ):
                t0 = j * IC
                pending = []
                if j > 0:
                    pending += d_emitters(j - 1)
                if j + 1 < NJ:
                    load_x(j + 1)
                    pending += b_emitters(j + 1)
                ngroups = FS * 4 * (j + 1)
                gi = 0
                emitted = 0

                # ---- C(j): attention for queries [t0, t0+512) ----
                for p in range(FS):  # head pair p = heads (2p, 2p+1)
                    njc = 4 * (j + 1)
                    ypA = psp.tile([65, IC], f32, tag="ypA", bufs=1)
                    ypB = psp.tile([65, IC], f32, tag="ypB", bufs=1)
                    pend = None  # (pt, w, o, jc) awaiting mask+PV

                    def flush_pv(pend):
                        pt, w, o, jc = pend
                        if o >= 0:
                            nc.vector.tensor_mul(pt[:, 0:JC], pt[:, 0:JC], tril_sb[:])
                            nc.vector.tensor_mul(
                                pt[:, w : w + JC], pt[:, w : w + JC], tril_sb[:]
                            )
                        for yp, h in ((ypA, 0), (ypB, 1)):
                            nc.tensor.matmul(
                                yp[:, IC - w : IC],
                                v1_sb[:, jc, (2 * p + h) * 66 : (2 * p + h) * 66 + 65],
                                pt[:, h * w : (h + 1) * w],
                                start=(jc == 0),
                                stop=(jc == njc - 1),
                            )

                    for jc in range(njc):
                        o = jc - 4 * j  # >=0: diagonal block tile
                        w = IC if o < 0 else IC - 128 * o
                        qlo = t0 + (IC - w)
                        sc = psp.tile([128, 2 * IC], f32, tag="sc", bufs=2)
                        for h in range(2):
                            nc.tensor.matmul(
                                sc[:, IC - w + h * w : IC + h * w],
                                kt_sb[h * 64 : h * 64 + 64, p, jc * JC : (jc + 1) * JC],
                                qt_sb[h * 64 : h * 64 + 64, p, qlo : t0 + IC],
                                start=True,
                                stop=True,
                            )
                        pt = ptp.tile([128, 2 * IC], bf, tag="pt")
                        # scale folds in the 2^-10 compensating the x32
                        # pre-scale applied to each of Wq and Wk (fp8 range)
                        nc.scalar.activation(
                            pt[:, 0 : 2 * w], sc[:, IC - w : IC + w], Act.Exp,
                            scale=0.125 / 1024.0,
                        )
                        if pend is not None:
                            flush_pv(pend)
                        elif norm_pending is not None:
                            # prev pair's normalization, emitted after this
                            # pair's first exp so QK/exp restart immediately
                            norm_pending()
                            norm_pending = None
                        pend = (pt, w, o, jc)
                        # spread next chunk's projection groups across this
                        # chunk's attention groups (fills PE exp-wait gaps)
                        gi += 1
                        while emitted < len(pending) and emitted * ngroups < gi * len(pending):
                            pending[emitted]()
                            emitted += 1
                    flush_pv(pend)

                    def make_norm(p=p, ypA=ypA, ypB=ypB):
                        # normalize: rows scaled by 1/denominator (yp row
                        # 64).  Broadcast both heads' reciprocal rows into
                        # one PSUM tile via col-packed K=1 matmuls, evacuate
                        # once, then scale each head's yp into yt.
                        def norm():
                            bc = psp.tile([128, IC], f32, tag="aux", bufs=2)
                            for yp, h in ((ypA, 0), (ypB, 1)):
                                rcr = work.tile([1, IC], bf, tag="rcr")
                                with nc.allow_low_precision(reason="softmax recip bf16"):
                                    nc.vector.reciprocal(rcr[:], yp[64:65, :])
                                nc.tensor.matmul(
                                    bc[h * DK : (h + 1) * DK, :], ones64[:], rcr[:],
                                    start=True, stop=True,
                                )
                            bcs = work.tile([128, IC], f32, tag="bcs")
                            nc.vector.tensor_copy(bcs[:], bc[:])
                            for yp, h in ((ypA, 0), (ypB, 1)):
                                nc.vector.tensor_mul(
                                    yt_sb[h * DK : (h + 1) * DK, p, t0 : t0 + IC],
                                    yp[0:DK, :],
                                    bcs[h * DK : (h + 1) * DK, :],
                                )
                        return norm

                    norm_pending = make_norm()
                if norm_pending is not None:
                    norm_pending()

            # final chunk's output projection (tail-optimized)
            for em in d_emitters(NJ - 1, tail=True):
                em()

            if debug_taps:
                nc.sync.dma_start(dbg_qt[:], qt_sb[:])
                nc.sync.dma_start(dbg_kt[:], kt_sb[:])
                nc.sync.dma_start(dbg_v1[:], v1_sb[:])
                nc.sync.dma_start(dbg_yt[:], yt_sb[:])

    nc.compile()
    return nc


def _get_program():
    if "nc" not in _CACHE:
        _CACHE["nc"] = _build_program()
    return _CACHE["nc"]


def _prepare_in_maps(inputs):
    import ml_dtypes

    bfd = ml_dtypes.bfloat16
    f8d = ml_dtypes.float8_e4m3
    WSCALE = 32.0  # q/k weights pre-scaled into fp8 normal range
    x = np.asarray(inputs["x"], dtype=np.float32)
    Wq = np.asarray(inputs["Wq"], dtype=np.float32)
    Wk = np.asarray(inputs["Wk"], dtype=np.float32)
    Wv = np.asarray(inputs["Wv"], dtype=np.float32)
    Wo = np.asarray(inputs["Wo"], dtype=np.float32)
    bq = np.asarray(inputs["bq"], dtype=np.float32)
    bk = np.asarray(inputs["bk"], dtype=np.float32)
    bv = np.asarray(inputs["bv"], dtype=np.float32)

    # valid iff key (partition) <= query (free): upper-triangular 0/1 mask
    tril = np.triu(np.ones((JC, JC), dtype=np.float32)).astype(bfd)
    xTb = [np.ascontiguousarray(x[b].reshape(T, E).T).astype(bfd) for b in range(B)]
    x8b = [np.ascontiguousarray(x[b].reshape(T, E).T).astype(f8d) for b in range(B)]

    in_maps = []
    for c in range(N_CORES):
        b, hg = c // 4, c % 4
        sl = slice(hg * F, (hg + 1) * F)
        wv_p = np.zeros((E, VW), dtype=bfd)
        bvr = np.zeros((1, VW), dtype=bfd)
        Wv_sl = Wv[sl]
        bv_sl = bv[sl]
        for h in range(NH_LOC):
            wv_p[:, h * 66 : h * 66 + 64] = Wv_sl[h * 64 : (h + 1) * 64].T.astype(bfd)
            bvr[0, h * 66 : h * 66 + 64] = bv_sl[h * 64 : (h + 1) * 64].astype(bfd)
            bvr[0, h * 66 + 64] = 1.0
        in_maps.append(
            {
                "xT": xTb[b],
                "x8": x8b[b],
                "wq": np.ascontiguousarray(Wq[sl].T * WSCALE).astype(f8d),
                "wk": np.ascontiguousarray(Wk[sl].T * WSCALE).astype(f8d),
                "wv": wv_p,
                "wo": np.ascontiguousarray(Wo[:, sl].T).astype(bfd),
                "bq": np.ascontiguousarray(bq[sl] * WSCALE),
                "bk": np.ascontiguousarray(bk[sl] * WSCALE),
                "bvr": bvr,
                "tril": tril,
            }
        )
    return in_maps


def kernel(x, Wq, bq, Wk, bk, Wv, bv, Wo, bo):
    from concourse.bass_utils import run_bass_kernel_spmd

    nc = _get_program()
    bo = np.asarray(bo, dtype=np.float32)
    in_maps = _prepare_in_maps(
        {"x": x, "Wq": Wq, "bq": bq, "Wk": Wk, "bk": bk,
         "Wv": Wv, "bv": bv, "Wo": Wo, "bo": bo}
    )

    res = run_bass_kernel_spmd(nc, in_maps, core_ids=list(range(N_CORES)))
    out = np.zeros((B, T, E), dtype=np.float64)
    for c in range(N_CORES):
        out[c // 4] += res.results[c]["partial"]
    out += bo[None, None, :]
    return out.astype(np.float32)


# revision 46
# speedup vs baseline: 1.4104x; 1.4104x over previous
"""Causal self-attention (B=2, T=2048, E=1024, H=16, d_k=64) on 8 TRN2 cores.

Hybrid sharding: core c owns batch c//4 and head group c%4 (4 heads =
feature slice 256*(c%4) .. 256*(c%4+1)).  Each core computes a partial
output [2048, 1024] for its batch; the host sums 4 partials per batch
and adds bo.

All matmuls run in bfloat16 (fp32 PSUM accumulation).  Per 512-token
chunk j the kernel pipelines: QKV projection (PE-dense), attention
(ACT-dense: one exp instruction covers both heads of a row-packed
QK pair), then the output projection.  Causal structure is exploited
by narrowing diagonal score tiles to >=their valid query range and
applying a 128x128 triangular 0/1 mask by multiply-after-exp.  The
softmax denominator comes from a ones column appended to V (computed
by the same ones-row matmul that adds the V bias).
"""

import numpy as np

B = 2
T = 2048          # tokens per batch (= per core)
E = 1024
F = 256           # per-core QKV features (4 heads x 64)
DK = 64
NH_LOC = 4        # heads per core
N_CORES = 8
IC = 512          # query chunk
JC = 128          # key chunk
NJ = T // IC      # 512-token chunks
N_EC = E // 128   # contraction chunks
FS = F // 128     # feature slices (partition groups)
VW = NH_LOC * 66  # padded V width: per head 64 feats + ones col + pad

_CACHE = {}


def _build_program(debug_taps=False):
    import concourse.mybir as mybir
    import concourse.tile as tile
    from concourse import bacc

    f32 = mybir.dt.float32
    bf = mybir.dt.bfloat16
    Act = mybir.ActivationFunctionType

    nc = bacc.Bacc("TRN2", target_bir_lowering=False, debug=False)

    f8 = mybir.dt.float8e4
    xT_ap = nc.dram_tensor("xT", [E, T], bf, kind="ExternalInput").ap()
    x8_ap = nc.dram_tensor("x8", [E, T], f8, kind="ExternalInput").ap()
    wq_ap = nc.dram_tensor("wq", [E, F], f8, kind="ExternalInput").ap()
    wk_ap = nc.dram_tensor("wk", [E, F], f8, kind="ExternalInput").ap()
    wv_ap = nc.dram_tensor("wv", [E, VW], bf, kind="ExternalInput").ap()
    wo_ap = nc.dram_tensor("wo", [F, E], bf, kind="ExternalInput").ap()
    bq_ap = nc.dram_tensor("bq", [F], f32, kind="ExternalInput").ap()
    bk_ap = nc.dram_tensor("bk", [F], f32, kind="ExternalInput").ap()
    bvr_ap = nc.dram_tensor("bvr", [1, VW], bf, kind="ExternalInput").ap()
    tril_ap = nc.dram_tensor("tril", [JC, JC], bf, kind="ExternalInput").ap()
    out_ap = nc.dram_tensor("partial", [T, E], f32, kind="ExternalOutput").ap()
    if debug_taps:
        dbg_qt = nc.dram_tensor("dbg_qt", [128, FS, T], bf, kind="ExternalOutput").ap()
        dbg_kt = nc.dram_tensor("dbg_kt", [128, FS, T], bf, kind="ExternalOutput").ap()
        dbg_v1 = nc.dram_tensor("dbg_v1", [128, T // JC, VW], bf, kind="ExternalOutput").ap()
        dbg_yt = nc.dram_tensor("dbg_yt", [128, FS, T], bf, kind="ExternalOutput").ap()

    with tile.TileContext(nc) as tc:
        with (
            tc.tile_pool(name="const", bufs=1) as constp,
            tc.tile_pool(name="persist", bufs=1) as persist,
            tc.tile_pool(name="xt", bufs=2) as xtp,
            tc.tile_pool(name="pt", bufs=4) as ptp,
            tc.tile_pool(name="work", bufs=3) as work,
            tc.tile_pool(name="ob", bufs=3) as obp,
            tc.tile_pool(name="ps", bufs=1, space="PSUM") as psp,
        ):
            # ---- constants (wq first: the first projection needs only it
            # and x chunk 0; the x load goes on the scalar HWDGE ring so it
            # runs in parallel with the weight loads on the SP ring) ----
            # q/k projection weights in fp8 (DoubleRow: 2 contraction
            # k-tiles of 128 per pass -> [p, pass, ktile, f])
            wq_sb = constp.tile([128, N_EC // 2, 2, F], f8, tag="wq")
            nc.gpsimd.dma_start(
                wq_sb[:], wq_ap.rearrange("(a i p) f -> p a i f", p=128, i=2)
            )
            bq_sb = constp.tile([128, FS], f32, tag="bq")
            nc.sync.dma_start(bq_sb[:], bq_ap.rearrange("(s p) -> p s", p=128))
            wk_sb = constp.tile([128, N_EC // 2, 2, F], f8, tag="wk")
            nc.sync.dma_start(
                wk_sb[:], wk_ap.rearrange("(a i p) f -> p a i f", p=128, i=2)
            )
            bk_sb = constp.tile([128, FS], f32, tag="bk")
            nc.sync.dma_start(bk_sb[:], bk_ap.rearrange("(s p) -> p s", p=128))
            wv_sb = constp.tile([128, N_EC, VW], bf, tag="wv")
            nc.sync.dma_start(wv_sb[:], wv_ap.rearrange("(a p) f -> p a f", p=128))
            bvr_sb = constp.tile([1, VW], bf, tag="bvr")
            nc.sync.dma_start(bvr_sb[:], bvr_ap)
            tril_sb = constp.tile([128, JC], bf, tag="tril")
            nc.sync.dma_start(tril_sb[:], tril_ap)
            wo_sb = constp.tile([128, FS, E], bf, tag="wo")
            nc.sync.dma_start(wo_sb[:], wo_ap.rearrange("(s p) e -> p s e", p=128))
            ones_r = constp.tile([1, JC], bf, tag="ones_r")
            nc.vector.memset(ones_r[:], 1.0)
            ones64 = constp.tile([1, DK], bf, tag="ones64")
            nc.vector.memset(ones64[:], 1.0)
            ones_f32 = constp.tile([128, 1], f32, tag="ones_f32")
            nc.vector.memset(ones_f32[:], 1.0)
            ones_row = ones_f32[:, 0:1].broadcast_to([128, IC])

            # ---- persistent activations ----
            qt_sb = persist.tile([128, FS, T], bf, tag="qt")      # [f, fs, t]
            kt_sb = persist.tile([128, FS, T], bf, tag="kt")
            v1_sb = persist.tile([128, T // JC, VW], bf, tag="v1")  # [t%128, kc, hf]
            yt_sb = persist.tile([128, FS, T], bf, tag="yt")

            xre = xT_ap.rearrange("(a p) t -> p a t", p=128)
            x8re = x8_ap.rearrange("(a i p) t -> p a i t", p=128, i=2)
            xts = [None] * NJ
            x8ts = [None] * NJ

            def load_x(j):
                # fp8 copy feeds the q/k projections (DoubleRow)
                x8t = xtp.tile([128, N_EC // 2, 2, IC], f8, tag="x8t", name=f"x8t{j}")
                nc.scalar.dma_start(x8t[:], x8re[:, :, :, j * IC : (j + 1) * IC])
                x8ts[j] = x8t
                # bf16 copy feeds the V projection
                xt = xtp.tile([128, N_EC, IC], bf, tag="xt", name=f"xt{j}")
                step = 4
                for e0 in range(0, N_EC, step):
                    nc.scalar.dma_start(
                        xt[:, e0 : e0 + step, :],
                        xre[:, e0 : e0 + step, j * IC : (j + 1) * IC],
                    )
                xts[j] = xt

            def b_emitters(j):
                """Per-psum-group emission closures for chunk j's QKV
                projection.  Interleaved into the previous chunk's
                (ACT-bound) attention phase to fill PE idle time."""
                t0 = j * IC
                xt = xts[j]
                x8t = x8ts[j]
                ems = []
                for w_sb, b_sb, dst in (
                    (wq_sb, bq_sb, qt_sb),
                    (wk_sb, bk_sb, kt_sb),
                ):
                    for fs in range(FS):
                        def em(w_sb=w_sb, b_sb=b_sb, dst=dst, fs=fs):
                            pq = psp.tile([128, IC], f32, tag="aux", bufs=2)
                            for a in range(N_EC // 2):
                                nc.tensor.matmul(
                                    pq[:],
                                    w_sb[:, a, :, fs * 128 : (fs + 1) * 128],
                                    x8t[:, a, :, :],
                                    start=(a == 0),
                                    stop=(a == N_EC // 2 - 1),
                                    perf_mode=mybir.MatmulPerfMode.DoubleRow,
                                )
                            # bias-add + fp32->bf16 move on DVE (ACT is the
                            # attention-phase bottleneck; keep it exp-only)
                            nc.vector.scalar_tensor_tensor(
                                dst[:, fs, t0 : t0 + IC], pq[:],
                                b_sb[:, fs : fs + 1], ones_row[:],
                                op0=mybir.AluOpType.add,
                                op1=mybir.AluOpType.mult,
                            )
                        ems.append(em)
                # V in [token, feat] layout; ones-row matmul adds bias AND
                # writes the per-head ones column (bvr has 1.0 there).
                for tsub in range(IC // 128):
                    def em(tsub=tsub):
                        pv = psp.tile([128, VW], f32, tag="aux", bufs=2)
                        for ec in range(N_EC):
                            nc.tensor.matmul(
                                pv[:],
                                xt[:, ec, tsub * 128 : (tsub + 1) * 128],
                                wv_sb[:, ec, :],
                                start=(ec == 0),
                                stop=False,
                            )
                        nc.tensor.matmul(
                            pv[:], ones_r[:], bvr_sb[:], start=False, stop=True
                        )
                        nc.vector.tensor_copy(v1_sb[:, j * 4 + tsub, :], pv[:])
                    ems.append(em)
                return ems

            def d_emitters(j, tail=False):
                """Per-128-token output-projection closures for chunk j.
                Interleaved into the NEXT chunk's attention phase; the final
                chunk's run at the end uses the idle ACT engine + HWDGE ring
                to shorten the drain tail."""
                t0 = j * IC
                ems = []
                for tsub in range(IC // 128):
                    def em(tsub=tsub):
                        tt = t0 + tsub * 128
                        ob = obp.tile([128, E], f32, tag="ob")
                        for eo in range(2):
                            od = psp.tile([128, 512], f32, tag="aux", bufs=2)
                            for fs in range(FS):
                                nc.tensor.matmul(
                                    od[:],
                                    yt_sb[:, fs, tt : tt + 128],
                                    wo_sb[:, fs, eo * 512 : (eo + 1) * 512],
                                    start=(fs == 0),
                                    stop=(fs == FS - 1),
                                )
                            cp_eng = nc.scalar if (tail and eo == 1) else nc.vector
                            if cp_eng is nc.scalar:
                                cp_eng.copy(ob[:, eo * 512 : (eo + 1) * 512], od[:])
                            else:
                                cp_eng.tensor_copy(
                                    ob[:, eo * 512 : (eo + 1) * 512], od[:]
                                )
                            if tail:
                                # split + HWDGE: drain the last chunk fast
                                nc.sync.dma_start(
                                    out_ap[tt : tt + 128, eo * 512 : (eo + 1) * 512],
                                    ob[:, eo * 512 : (eo + 1) * 512],
                                )
                        if not tail:
                            # ride the idle GpSimd SWDGE queue mid-kernel
                            nc.gpsimd.dma_start(out_ap[tt : tt + 128, :], ob[:])
                    ems.append(em)
                return ems

            load_x(0)
            for em in b_emitters(0):
                em()
            norm_pending = None  # prev pair's normalization closure
            for j in range(NJ):
                t0 = j * IC
                pending = []
                if j > 0:
                    pending += d_emitters(j - 1)
                if j + 1 < NJ:
                    load_x(j + 1)
                    pending += b_emitters(j + 1)
                ngroups = FS * 4 * (j + 1)
                gi = 0
                emitted = 0

                # ---- C(j): attention for queries [t0, t0+512) ----
                for p in range(FS):  # head pair p = heads (2p, 2p+1)
                    njc = 4 * (j + 1)
                    ypA = psp.tile([65, IC], f32, tag="ypA", bufs=1)
                    ypB = psp.tile([65, IC], f32, tag="ypB", bufs=1)
                    pend = None  # (pt, w, o, jc) awaiting mask+PV

                    def flush_pv(pend):
                        pt, w, o, jc = pend
                        if o >= 0:
                            nc.vector.tensor_mul(pt[:, 0:JC], pt[:, 0:JC], tril_sb[:])
                            nc.vector.tensor_mul(
                                pt[:, w : w + JC], pt[:, w : w + JC], tril_sb[:]
                            )
                        for yp, h in ((ypA, 0), (ypB, 1)):
                            nc.tensor.matmul(
                                yp[:, IC - w : IC],
                                v1_sb[:, jc, (2 * p + h) * 66 : (2 * p + h) * 66 + 65],
                                pt[:, h * w : (h + 1) * w],
                                start=(jc == 0),
                                stop=(jc == njc - 1),
                            )

                    for jc in range(njc):
                        o = jc - 4 * j  # >=0: diagonal block tile
                        w = IC if o < 0 else IC - 128 * o
                        qlo = t0 + (IC - w)
                        sc = psp.tile([128, 2 * IC], f32, tag="sc", bufs=2)
                        for h in range(2):
                            nc.tensor.matmul(
                                sc[:, IC - w + h * w : IC + h * w],
                                kt_sb[h * 64 : h * 64 + 64, p, jc * JC : (jc + 1) * JC],
                                qt_sb[h * 64 : h * 64 + 64, p, qlo : t0 + IC],
                                start=True,
                                stop=True,
                            )
                        pt = ptp.tile([128, 2 * IC], bf, tag="pt")
                        # scale folds in the 2^-10 compensating the x32
                        # pre-scale applied to each of Wq and Wk (fp8 range)
                        nc.scalar.activation(
                            pt[:, 0 : 2 * w], sc[:, IC - w : IC + w], Act.Exp,
                            scale=0.125 / 1024.0,
                        )
                        if pend is not None:
                            flush_pv(pend)
                        elif norm_pending is not None:
                            # prev pair's normalization, emitted after this
                            # pair's first exp so QK/exp restart immediately
                            norm_pending()
                            norm_pending = None
                        pend = (pt, w, o, jc)
                        # spread next chunk's projection groups across this
                        # chunk's attention groups (fills PE exp-wait gaps)
                        gi += 1
                        while emitted < len(pending) and emitted * ngroups < gi * len(pending):
                            pending[emitted]()
                            emitted += 1
                    flush_pv(pend)

                    def make_norm(p=p, ypA=ypA, ypB=ypB, t0=t0):
                        # normalize: rows scaled by 1/denominator (yp row
                        # 64).  Broadcast both heads' reciprocal rows into
                        # one PSUM tile via col-packed K=1 matmuls, evacuate
                        # once, then scale each head's yp into yt.
                        def norm():
                            bc = psp.tile([128, IC], f32, tag="aux", bufs=2)
                            for yp, h in ((ypA, 0), (ypB, 1)):
                                rcr = work.tile([1, IC], bf, tag="rcr")
                                with nc.allow_low_precision(reason="softmax recip bf16"):
                                    nc.vector.reciprocal(rcr[:], yp[64:65, :])
                                nc.tensor.matmul(
                                    bc[h * DK : (h + 1) * DK, :], ones64[:], rcr[:],
                                    start=True, stop=True,
                                )
                            bcs = work.tile([128, IC], f32, tag="bcs")
                            nc.vector.tensor_copy(bcs[:], bc[:])
                            for yp, h in ((ypA, 0), (ypB, 1)):
                                nc.vector.tensor_mul(
                                    yt_sb[h * DK : (h + 1) * DK, p, t0 : t0 + IC],
                                    yp[0:DK, :],
                                    bcs[h * DK : (h + 1) * DK, :],
                                )
                        return norm

                    norm_pending = make_norm()

            if norm_pending is not None:
                norm_pending()
            # final chunk's output projection (tail-optimized)
            for em in d_emitters(NJ - 1, tail=True):
                em()

            if debug_taps:
                nc.sync.dma_start(dbg_qt[:], qt_sb[:])
                nc.sync.dma_start(dbg_kt[:], kt_sb[:])
                nc.sync.dma_start(dbg_v1[:], v1_sb[:])
                nc.sync.dma_start(dbg_yt[:], yt_sb[:])

    nc.compile()
    return nc


def _get_program():
    if "nc" not in _CACHE:
        _CACHE["nc"] = _build_program()
    return _CACHE["nc"]


def _prepare_in_maps(inputs):
    import ml_dtypes

    bfd = ml_dtypes.bfloat16
    f8d = ml_dtypes.float8_e4m3
    WSCALE = 32.0  # q/k weights pre-scaled into fp8 normal range
    x = np.asarray(inputs["x"], dtype=np.float32)
    Wq = np.asarray(inputs["Wq"], dtype=np.float32)
    Wk = np.asarray(inputs["Wk"], dtype=np.float32)
    Wv = np.asarray(inputs["Wv"], dtype=np.float32)
    Wo = np.asarray(inputs["Wo"], dtype=np.float32)
    bq = np.asarray(inputs["bq"], dtype=np.float32)
    bk = np.asarray(inputs["bk"], dtype=np.float32)
    bv = np.asarray(inputs["bv"], dtype=np.float32)

    # valid iff key (partition) <= query (free): upper-triangular 0/1 mask
    tril = np.triu(np.ones((JC, JC), dtype=np.float32)).astype(bfd)
    xTb = [np.ascontiguousarray(x[b].reshape(T, E).T).astype(bfd) for b in range(B)]
    x8b = [np.ascontiguousarray(x[b].reshape(T, E).T).astype(f8d) for b in range(B)]

    in_maps = []
    for c in range(N_CORES):
        b, hg = c // 4, c % 4
        sl = slice(hg * F, (hg + 1) * F)
        wv_p = np.zeros((E, VW), dtype=bfd)
        bvr = np.zeros((1, VW), dtype=bfd)
        Wv_sl = Wv[sl]
        bv_sl = bv[sl]
        for h in range(NH_LOC):
            wv_p[:, h * 66 : h * 66 + 64] = Wv_sl[h * 64 : (h + 1) * 64].T.astype(bfd)
            bvr[0, h * 66 : h * 66 + 64] = bv_sl[h * 64 : (h + 1) * 64].astype(bfd)
            bvr[0, h * 66 + 64] = 1.0
        in_maps.append(
            {
                "xT": xTb[b],
                "x8": x8b[b],
                "wq": np.ascontiguousarray(Wq[sl].T * WSCALE).astype(f8d),
                "wk": np.ascontiguousarray(Wk[sl].T * WSCALE).astype(f8d),
                "wv": wv_p,
                "wo": np.ascontiguousarray(Wo[:, sl].T).astype(bfd),
                "bq": np.ascontiguousarray(bq[sl] * WSCALE),
                "bk": np.ascontiguousarray(bk[sl] * WSCALE),
                "bvr": bvr,
                "tril": tril,
            }
        )
    return in_maps


def kernel(x, Wq, bq, Wk, bk, Wv, bv, Wo, bo):
    from concourse.bass_utils import run_bass_kernel_spmd

    nc = _get_program()
    bo = np.asarray(bo, dtype=np.float32)
    in_maps = _prepare_in_maps(
        {"x": x, "Wq": Wq, "bq": bq, "Wk": Wk, "bk": bk,
         "Wv": Wv, "bv": bv, "Wo": Wo, "bo": bo}
    )

    res = run_bass_kernel_spmd(nc, in_maps, core_ids=list(range(N_CORES)))
    out = np.zeros((B, T, E), dtype=np.float64)
    for c in range(N_CORES):
        out[c // 4] += res.results[c]["partial"]
    out += bo[None, None, :]
    return out.astype(np.float32)


# revision 49
# speedup vs baseline: 2.0601x; 1.4607x over previous
"""Causal self-attention (B=2, T=2048, E=1024, H=16, d_k=64) on 8 TRN2 cores.

Hybrid sharding: core c owns batch c//4 and head group c%4 (4 heads =
feature slice 256*(c%4) .. 256*(c%4+1)).  Each core computes a partial
output [2048, 1024] for its batch; the host sums 4 partials per batch
and adds bo.

Matmuls run in bfloat16 (fp32 PSUM accumulation), except the Q/K
projections which use fp8e4m3 with DoubleRow perf mode (256-deep
contraction per pass, 2x throughput; weights pre-scaled x32 into the
fp8 normal range, compensated inside the softmax exp scale).  V and
the output projection stay bf16 so output precision is dominated by
bf16 (~1.1e-2 rel err vs the 2e-2 gate).

Per 512-token chunk j the kernel pipelines: QKV projection (PE-dense),
attention (ACT-dense: one exp instruction covers both heads of a
row-packed QK pair), then the output projection.  The projection work
of chunk j+1 and the output projection of chunk j-1 are interleaved
into chunk j's ACT-bound attention groups to fill PE idle time.
Causal structure is exploited by narrowing diagonal score tiles to
their valid query range and applying a 128x128 triangular 0/1 mask by
multiply-after-exp.  The softmax denominator comes from a ones column
appended to V (written by the same ones-row matmul that adds the V
bias).  Out-DMAs ride the GpSimd SWDGE queue; x/weight loads use the
two HWDGE rings.
"""

import numpy as np

B = 2
T = 2048          # tokens per batch (= per core)
E = 1024
F = 256           # per-core QKV features (4 heads x 64)
DK = 64
NH_LOC = 4        # heads per core
N_CORES = 8
IC = 512          # query chunk
JC = 128          # key chunk
NJ = T // IC      # 512-token chunks
N_EC = E // 128   # contraction chunks
FS = F // 128     # feature slices (partition groups)
VW = NH_LOC * 66  # padded V width: per head 64 feats + ones col + pad

_CACHE = {}


def _build_program(debug_taps=False):
    import concourse.mybir as mybir
    import concourse.tile as tile
    from concourse import bacc

    f32 = mybir.dt.float32
    bf = mybir.dt.bfloat16
    Act = mybir.ActivationFunctionType

    nc = bacc.Bacc("TRN2", target_bir_lowering=False, debug=False)

    f8 = mybir.dt.float8e4
    xT_ap = nc.dram_tensor("xT", [E, T], bf, kind="ExternalInput").ap()
    x8_ap = nc.dram_tensor("x8", [E, T], f8, kind="ExternalInput").ap()
    wq_ap = nc.dram_tensor("wq", [E, F], f8, kind="ExternalInput").ap()
    wk_ap = nc.dram_tensor("wk", [E, F], f8, kind="ExternalInput").ap()
    wv_ap = nc.dram_tensor("wv", [E, VW], bf, kind="ExternalInput").ap()
    wo_ap = nc.dram_tensor("wo", [F, E], bf, kind="ExternalInput").ap()
    bq_ap = nc.dram_tensor("bq", [F], f32, kind="ExternalInput").ap()
    bk_ap = nc.dram_tensor("bk", [F], f32, kind="ExternalInput").ap()
    bvr_ap = nc.dram_tensor("bvr", [1, VW], bf, kind="ExternalInput").ap()
    tril_ap = nc.dram_tensor("tril", [JC, JC], bf, kind="ExternalInput").ap()
    out_ap = nc.dram_tensor("partial", [T, E], f32, kind="ExternalOutput").ap()
    if debug_taps:
        dbg_qt = nc.dram_tensor("dbg_qt", [128, FS, T], bf, kind="ExternalOutput").ap()
        dbg_kt = nc.dram_tensor("dbg_kt", [128, FS, T], bf, kind="ExternalOutput").ap()
        dbg_v1 = nc.dram_tensor("dbg_v1", [128, T // JC, VW], bf, kind="ExternalOutput").ap()
        dbg_yt = nc.dram_tensor("dbg_yt", [128, FS, T], bf, kind="ExternalOutput").ap()

    with tile.TileContext(nc) as tc:
        with (
            tc.tile_pool(name="const", bufs=1) as constp,
            tc.tile_pool(name="persist", bufs=1) as persist,
            tc.tile_pool(name="xt", bufs=2) as xtp,
            tc.tile_pool(name="pt", bufs=4) as ptp,
            tc.tile_pool(name="work", bufs=3) as work,
            tc.tile_pool(name="ob", bufs=3) as obp,
            tc.tile_pool(name="ps", bufs=1, space="PSUM") as psp,
        ):
            # ---- constants (wq first: the first projection needs only it
            # and x chunk 0; the x load goes on the scalar HWDGE ring so it
            # runs in parallel with the weight loads on the SP ring) ----
            # q/k projection weights in fp8 (DoubleRow: 2 contraction
            # k-tiles of 128 per pass -> [p, pass, ktile, f])
            wq_sb = constp.tile([128, N_EC // 2, 2, F], f8, tag="wq")
            nc.sync.dma_start(
                wq_sb[:], wq_ap.rearrange("(a i p) f -> p a i f", p=128, i=2)
            )
            bq_sb = constp.tile([128, FS], f32, tag="bq")
            nc.sync.dma_start(bq_sb[:], bq_ap.rearrange("(s p) -> p s", p=128))
            wk_sb = constp.tile([128, N_EC // 2, 2, F], f8, tag="wk")
            nc.sync.dma_start(
                wk_sb[:], wk_ap.rearrange("(a i p) f -> p a i f", p=128, i=2)
            )
            bk_sb = constp.tile([128, FS], f32, tag="bk")
            nc.sync.dma_start(bk_sb[:], bk_ap.rearrange("(s p) -> p s", p=128))
            wv_sb = constp.tile([128, N_EC, VW], bf, tag="wv")
            nc.sync.dma_start(wv_sb[:], wv_ap.rearrange("(a p) f -> p a f", p=128))
            bvr_sb = constp.tile([1, VW], bf, tag="bvr")
            nc.sync.dma_start(bvr_sb[:], bvr_ap)
            tril_sb = constp.tile([128, JC], bf, tag="tril")
            nc.sync.dma_start(tril_sb[:], tril_ap)
            wo_sb = constp.tile([128, FS, E], bf, tag="wo")
            nc.sync.dma_start(wo_sb[:], wo_ap.rearrange("(s p) e -> p s e", p=128))
            ones_r = constp.tile([1, JC], bf, tag="ones_r")
            nc.vector.memset(ones_r[:], 1.0)
            ones64 = constp.tile([1, DK], bf, tag="ones64")
            nc.vector.memset(ones64[:], 1.0)
            ones_f32 = constp.tile([128, 1], f32, tag="ones_f32")
            nc.vector.memset(ones_f32[:], 1.0)
            ones_row = ones_f32[:, 0:1].broadcast_to([128, IC])

            # ---- persistent activations ----
            qt_sb = persist.tile([128, FS, T], bf, tag="qt")      # [f, fs, t]
            kt_sb = persist.tile([128, FS, T], bf, tag="kt")
            v1_sb = persist.tile([128, T // JC, VW], bf, tag="v1")  # [t%128, kc, hf]
            yt_sb = persist.tile([128, FS, T], bf, tag="yt")

            xre = xT_ap.rearrange("(a p) t -> p a t", p=128)
            x8re = x8_ap.rearrange("(a i p) t -> p a i t", p=128, i=2)
            xts = [None] * NJ
            x8ts = [None] * NJ

            def load_x(j):
                # fp8 copy feeds the q/k projections (DoubleRow)
                x8t = xtp.tile([128, N_EC // 2, 2, IC], f8, tag="x8t", name=f"x8t{j}")
                nc.scalar.dma_start(x8t[:], x8re[:, :, :, j * IC : (j + 1) * IC])
                x8ts[j] = x8t
                # bf16 copy feeds the V projection
                xt = xtp.tile([128, N_EC, IC], bf, tag="xt", name=f"xt{j}")
                step = 4
                for e0 in range(0, N_EC, step):
                    nc.scalar.dma_start(
                        xt[:, e0 : e0 + step, :],
                        xre[:, e0 : e0 + step, j * IC : (j + 1) * IC],
                    )
                xts[j] = xt

            def b_emitters(j):
                """Per-psum-group emission closures for chunk j's QKV
                projection.  Interleaved into the previous chunk's
                (ACT-bound) attention phase to fill PE idle time."""
                t0 = j * IC
                xt = xts[j]
                x8t = x8ts[j]
                ems = []
                for w_sb, b_sb, dst in (
                    (wq_sb, bq_sb, qt_sb),
                    (wk_sb, bk_sb, kt_sb),
                ):
                    for fs in range(FS):
                        def em(w_sb=w_sb, b_sb=b_sb, dst=dst, fs=fs):
                            pq = psp.tile([128, IC], f32, tag="aux", bufs=2)
                            for a in range(N_EC // 2):
                                nc.tensor.matmul(
                                    pq[:],
                                    w_sb[:, a, :, fs * 128 : (fs + 1) * 128],
                                    x8t[:, a, :, :],
                                    start=(a == 0),
                                    stop=(a == N_EC // 2 - 1),
                                    perf_mode=mybir.MatmulPerfMode.DoubleRow,
                                )
                            # bias-add + fp32->bf16 move on DVE (ACT is the
                            # attention-phase bottleneck; keep it exp-only)
                            nc.vector.scalar_tensor_tensor(
                                dst[:, fs, t0 : t0 + IC], pq[:],
                                b_sb[:, fs : fs + 1], ones_row[:],
                                op0=mybir.AluOpType.add,
                                op1=mybir.AluOpType.mult,
                            )
                        ems.append(em)
                # V in [token, feat] layout; ones-row matmul adds bias AND
                # writes the per-head ones column (bvr has 1.0 there).
                for tsub in range(IC // 128):
                    def em(tsub=tsub):
                        pv = psp.tile([128, VW], f32, tag="aux", bufs=2)
                        for ec in range(N_EC):
                            nc.tensor.matmul(
                                pv[:],
                                xt[:, ec, tsub * 128 : (tsub + 1) * 128],
                                wv_sb[:, ec, :],
                                start=(ec == 0),
                                stop=False,
                            )
                        nc.tensor.matmul(
                            pv[:], ones_r[:], bvr_sb[:], start=False, stop=True
                        )
                        nc.vector.tensor_copy(v1_sb[:, j * 4 + tsub, :], pv[:])
                    ems.append(em)
                return ems

            def d_emitters(j, tail=False):
                """Per-128-token output-projection closures for chunk j.
                Interleaved into the NEXT chunk's attention phase; the final
                chunk's run at the end uses the idle ACT engine + HWDGE ring
                to shorten the drain tail."""
                t0 = j * IC
                ems = []
                for tsub in range(IC // 128):
                    def em(tsub=tsub):
                        tt = t0 + tsub * 128
                        ob = obp.tile([128, E], f32, tag="ob")
                        for eo in range(2):
                            od = psp.tile([128, 512], f32, tag="aux", bufs=2)
                            for fs in range(FS):
                                nc.tensor.matmul(
                                    od[:],
                                    yt_sb[:, fs, tt : tt + 128],
                                    wo_sb[:, fs, eo * 512 : (eo + 1) * 512],
                                    start=(fs == 0),
                                    stop=(fs == FS - 1),
                                )
                            cp_eng = nc.scalar if (tail and eo == 1) else nc.vector
                            if cp_eng is nc.scalar:
                                cp_eng.copy(ob[:, eo * 512 : (eo + 1) * 512], od[:])
                            else:
                                cp_eng.tensor_copy(
                                    ob[:, eo * 512 : (eo + 1) * 512], od[:]
                                )
                            if tail:
                                # split + HWDGE: drain the last chunk fast
                                nc.sync.dma_start(
                                    out_ap[tt : tt + 128, eo * 512 : (eo + 1) * 512],
                                    ob[:, eo * 512 : (eo + 1) * 512],
                                )
                        if not tail:
                            # ride the idle GpSimd SWDGE queue mid-kernel
                            nc.gpsimd.dma_start(out_ap[tt : tt + 128, :], ob[:])
                    ems.append(em)
                return ems

            load_x(0)
            for em in b_emitters(0):
                em()
            norm_pending = None  # prev pair's normalization closure
            for j in range(NJ):
                t0 = j * IC
                pending = []
                if j > 0:
                    pending += d_emitters(j - 1)
                if j + 1 < NJ:
                    load_x(j + 1)
                    pending += b_emitters(j + 1)
                ngroups = FS * 4 * (j + 1)
                gi = 0
                emitted = 0

                # ---- C(j): attention for queries [t0, t0+512) ----
                for p in range(FS):  # head pair p = heads (2p, 2p+1)
                    njc = 4 * (j + 1)
                    ypA = psp.tile([65, IC], f32, tag="ypA", bufs=1)
                    ypB = psp.tile([65, IC], f32, tag="ypB", bufs=1)
                    pend = None  # (pt, w, o, jc) awaiting mask+PV

                    def flush_pv(pend):
                        pt, w, o, jc = pend
                        if o >= 0:
                            nc.vector.tensor_mul(pt[:, 0:JC], pt[:, 0:JC], tril_sb[:])
                            nc.vector.tensor_mul(
                                pt[:, w : w + JC], pt[:, w : w + JC], tril_sb[:]
                            )
                        for yp, h in ((ypA, 0), (ypB, 1)):
                            nc.tensor.matmul(
                                yp[:, IC - w : IC],
                                v1_sb[:, jc, (2 * p + h) * 66 : (2 * p + h) * 66 + 65],
                                pt[:, h * w : (h + 1) * w],
                                start=(jc == 0),
                                stop=(jc == njc - 1),
                            )

                    for jc in range(njc):
                        o = jc - 4 * j  # >=0: diagonal block tile
                        w = IC if o < 0 else IC - 128 * o
                        qlo = t0 + (IC - w)
                        sc = psp.tile([128, 2 * IC], f32, tag="sc", bufs=2)
                        for h in range(2):
                            nc.tensor.matmul(
                                sc[:, IC - w + h * w : IC + h * w],
                                kt_sb[h * 64 : h * 64 + 64, p, jc * JC : (jc + 1) * JC],
                                qt_sb[h * 64 : h * 64 + 64, p, qlo : t0 + IC],
                                start=True,
                                stop=True,
                            )
                        pt = ptp.tile([128, 2 * IC], bf, tag="pt")
                        # scale folds in the 2^-10 compensating the x32
                        # pre-scale applied to each of Wq and Wk (fp8 range)
                        nc.scalar.activation(
                            pt[:, 0 : 2 * w], sc[:, IC - w : IC + w], Act.Exp,
                            scale=0.125 / 1024.0,
                        )
                        if pend is not None:
                            flush_pv(pend)
                        elif norm_pending is not None:
                            # prev pair's normalization, emitted after this
                            # pair's first exp so QK/exp restart immediately
                            norm_pending()
                            norm_pending = None
                        pend = (pt, w, o, jc)
                        # spread next chunk's projection groups across this
                        # chunk's attention groups (fills PE exp-wait gaps)
                        gi += 1
                        while emitted < len(pending) and emitted * ngroups < gi * len(pending):
                            pending[emitted]()
                            emitted += 1
                    flush_pv(pend)

                    def make_norm(p=p, ypA=ypA, ypB=ypB, t0=t0):
                        # normalize: rows scaled by 1/denominator (yp row
                        # 64).  Broadcast both heads' reciprocal rows into
                        # one PSUM tile via col-packed K=1 matmuls, evacuate
                        # once, then scale each head's yp into yt.
                        def norm():
                            bc = psp.tile([128, IC], f32, tag="aux", bufs=2)
                            for yp, h in ((ypA, 0), (ypB, 1)):
                                rcr = work.tile([1, IC], bf, tag="rcr")
                                with nc.allow_low_precision(reason="softmax recip bf16"):
                                    nc.vector.reciprocal(rcr[:], yp[64:65, :])
                                nc.tensor.matmul(
                                    bc[h * DK : (h + 1) * DK, :], ones64[:], rcr[:],
                                    start=True, stop=True,
                                )
                            bcs = work.tile([128, IC], f32, tag="bcs")
                            nc.vector.tensor_copy(bcs[:], bc[:])
                            for yp, h in ((ypA, 0), (ypB, 1)):
                                nc.vector.tensor_mul(
                                    yt_sb[h * DK : (h + 1) * DK, p, t0 : t0 + IC],
                                    yp[0:DK, :],
                                    bcs[h * DK : (h + 1) * DK, :],
                                )
                        return norm

                    norm_pending = make_norm()

            if norm_pending is not None:
                norm_pending()
            # final chunk's output projection (tail-optimized)
            for em in d_emitters(NJ - 1, tail=True):
                em()

            if debug_taps:
                nc.sync.dma_start(dbg_qt[:], qt_sb[:])
                nc.sync.dma_start(dbg_kt[:], kt_sb[:])
                nc.sync.dma_start(dbg_v1[:], v1_sb[:])
                nc.sync.dma_start(dbg_yt[:], yt_sb[:])

    nc.compile()
    return nc


def _get_program():
    if "nc" not in _CACHE:
        _CACHE["nc"] = _build_program()
    return _CACHE["nc"]


def _prepare_in_maps(inputs):
    import ml_dtypes

    bfd = ml_dtypes.bfloat16
    f8d = ml_dtypes.float8_e4m3
    WSCALE = 32.0  # q/k weights pre-scaled into fp8 normal range
    x = np.asarray(inputs["x"], dtype=np.float32)
    Wq = np.asarray(inputs["Wq"], dtype=np.float32)
    Wk = np.asarray(inputs["Wk"], dtype=np.float32)
    Wv = np.asarray(inputs["Wv"], dtype=np.float32)
    Wo = np.asarray(inputs["Wo"], dtype=np.float32)
    bq = np.asarray(inputs["bq"], dtype=np.float32)
    bk = np.asarray(inputs["bk"], dtype=np.float32)
    bv = np.asarray(inputs["bv"], dtype=np.float32)

    # valid iff key (partition) <= query (free): upper-triangular 0/1 mask
    tril = np.triu(np.ones((JC, JC), dtype=np.float32)).astype(bfd)
    xTb = [np.ascontiguousarray(x[b].reshape(T, E).T).astype(bfd) for b in range(B)]
    x8b = [np.ascontiguousarray(x[b].reshape(T, E).T).astype(f8d) for b in range(B)]

    in_maps = []
    for c in range(N_CORES):
        b, hg = c // 4, c % 4
        sl = slice(hg * F, (hg + 1) * F)
        wv_p = np.zeros((E, VW), dtype=bfd)
        bvr = np.zeros((1, VW), dtype=bfd)
        Wv_sl = Wv[sl]
        bv_sl = bv[sl]
        for h in range(NH_LOC):
            wv_p[:, h * 66 : h * 66 + 64] = Wv_sl[h * 64 : (h + 1) * 64].T.astype(bfd)
            bvr[0, h * 66 : h * 66 + 64] = bv_sl[h * 64 : (h + 1) * 64].astype(bfd)
            bvr[0, h * 66 + 64] = 1.0
        in_maps.append(
            {
                "xT": xTb[b],
                "x8": x8b[b],
                "wq": np.ascontiguousarray(Wq[sl].T * WSCALE).astype(f8d),
                "wk": np.ascontiguousarray(Wk[sl].T * WSCALE).astype(f8d),
                "wv": wv_p,
                "wo": np.ascontiguousarray(Wo[:, sl].T).astype(bfd),
                "bq": np.ascontiguousarray(bq[sl] * WSCALE),
                "bk": np.ascontiguousarray(bk[sl] * WSCALE),
                "bvr": bvr,
                "tril": tril,
            }
        )
    return in_maps


def kernel(x, Wq, bq, Wk, bk, Wv, bv, Wo, bo):
    from concourse.bass_utils import run_bass_kernel_spmd

    nc = _get_program()
    bo = np.asarray(bo, dtype=np.float32)
    in_maps = _prepare_in_maps(
        {"x": x, "Wq": Wq, "bq": bq, "Wk": Wk, "bk": bk,
         "Wv": Wv, "bv": bv, "Wo": Wo, "bo": bo}
    )

    res = run_bass_kernel_spmd(nc, in_maps, core_ids=list(range(N_CORES)))
    out = np.zeros((B, T, E), dtype=np.float64)
    for c in range(N_CORES):
        out[c // 4] += res.results[c]["partial"]
    out += bo[None, None, :]
    return out.astype(np.float32)
